# revision 15
# baseline (speedup 1.0000x reference)
"""Distributed Trainium2 Bass kernel for the AttentionDecoder step (8 cores).

Sharding (batch=1):
- Embedding table sharded over hidden dim: core c holds emb[:, c*128:(c+1)*128]
  ([V*128, 1] bf16) and gathers its slice of row x via indirect DMA with
  on-device index arithmetic (idx = x*128 + p).
- Attention scores and the combine matvec are contraction-sharded: each core
  computes partials against its slice of the contraction dim; AllReduce-add
  yields the full vector on every core.
- LSTM is output-sharded: core c computes hidden units [c*128,(c+1)*128) of both
  directions; AllGather assembles (hf|hb|cf|cb).
- Output projection is vocab-sharded (bf16): core c computes logits for padded
  vocab [c*6656,(c+1)*6656); log-softmax normalizer via AllGather of per-core
  sum(exp(logits)).

All cores run one SPMD graph; every rank-dependent slice is realized through
per-core uploaded shard data or one-hot selector matrices (used as the rhs of
PE transposes).
"""
import numpy as np

import concourse.bass as bass
import concourse.mybir as mybir
import concourse.tile as tile
from concourse.masks import make_identity

H = 1024
V = 50257
L = 2048
NC = 8
VPC = 6656          # padded vocab per core (52*128 = 13*512)
VP = VPC * NC       # 53248
NEG = -1.0e30

F32 = mybir.dt.float32
BF16 = mybir.dt.bfloat16
I32 = mybir.dt.int32
AF = mybir.ActivationFunctionType
ALU = mybir.AluOpType
AX = mybir.AxisListType

CORE_IDS = list(range(NC))


def split_multiwait(nc, limit=1):
    """walrus in this image supports only `limit` sync-waits per instruction;
    hoist extras onto dedicated wait ops just before the instruction."""
    n = 0
    for func in nc.m.functions:
        for bb in func.blocks:
            new = []
            for inst in bb.instructions:
                si = inst.sync_info
                if si is not None and si.on_wait and len(si.on_wait) > limit:
                    waits = list(si.on_wait)
                    si.on_wait = waits[:limit]
                    for i, w in enumerate(waits[limit:]):
                        n += 1
                        new.append(mybir.InstEventSemaphore(
                            name=f"{inst.name}_wsplit{i}", opcode="EventSemaphore",
                            engine=inst.engine, ins=[], outs=[],
                            sync_info=mybir.SyncInfo(on_wait=[w], on_update=[])))
                new.append(inst)
            bb.instructions[:] = new
    return n


def build_bass(split=True):
    nc = bass.Bass()

    # ---- kernel I/O ------------------------------------------------------
    x128 = nc.dram_tensor("x128", [128, 1], I32, kind="ExternalInput")
    idxc = nc.dram_tensor("idxc", [128, 2], I32, kind="ExternalInput")
    emb_sh = nc.dram_tensor("emb_sh", [V * 128, 1], BF16, kind="ExternalInput")
    h0r = nc.dram_tensor("h0r", [16, 128], F32, kind="ExternalInput")
    c0_sh = nc.dram_tensor("c0_sh", [1, 256], F32, kind="ExternalInput")
    sel_in = nc.dram_tensor("sel_in", [16, 4], F32, kind="ExternalInput")
    attn_wT_sh = nc.dram_tensor("attn_wT_sh", [128, 4096], BF16, kind="ExternalInput")
    attn_b2 = nc.dram_tensor("attn_b2", [16, 128], F32, kind="ExternalInput")
    enc_sh = nc.dram_tensor("enc_sh", [128, 4096], BF16, kind="ExternalInput")
    comb_wT_sh = nc.dram_tensor("comb_wT_sh", [128, 3072], BF16, kind="ExternalInput")
    comb_b2 = nc.dram_tensor("comb_b2", [8, 128], F32, kind="ExternalInput")
    lstm_w = nc.dram_tensor("lstm_w", [128, 16384], BF16, kind="ExternalInput")
    lstm_b = nc.dram_tensor("lstm_b", [1, 2048], F32, kind="ExternalInput")
    wout = nc.dram_tensor("wout", [13, 128, 8192], BF16, kind="ExternalInput")
    outb_sh = nc.dram_tensor("outb_sh", [52, 128], F32, kind="ExternalInput")

    out_logp = nc.dram_tensor("out_logp", [VPC], F32, kind="ExternalOutput")
    out_att = nc.dram_tensor("out_att", [L], F32, kind="ExternalOutput")
    out_hc = nc.dram_tensor("out_hc", [8, 512], F32, kind="ExternalOutput")

    with tile.TileContext(nc) as tc:
        with (
            tc.tile_pool(name="sb", bufs=1) as sb,
            tc.tile_pool(name="wfix", bufs=1) as wfix,
            tc.tile_pool(name="wbig", bufs=5) as wbig,
            tc.tile_pool(name="dram", bufs=1, space="DRAM") as dram,
        ):
            # ---- internal DRAM (collective bounces + scratch) -----------
            cc_sc_in = dram.tile([L], F32, name="cc_sc_in")
            cc_sc_out = dram.tile([L], F32, addr_space="Shared", name="cc_sc_out")
            cc_aa_in = dram.tile([L], F32, name="cc_aa_in")
            cc_aa_out = dram.tile([L], F32, addr_space="Shared", name="cc_aa_out")
            cc_li_in = dram.tile([H], F32, name="cc_li_in")
            cc_li_out = dram.tile([H], F32, addr_space="Shared", name="cc_li_out")
            cc_hc_in = dram.tile([512], F32, name="cc_hc_in")
            cc_hc_out = dram.tile([8, 512], F32, addr_space="Shared", name="cc_hc_out")
            cc_st_in = dram.tile([8], F32, name="cc_st_in")
            cc_st_out = dram.tile([8, 8], F32, addr_space="Shared", name="cc_st_out")

            # ---- constants ----------------------------------------------
            ident = sb.tile([128, 128], F32, name="ident")
            make_identity(nc, ident[:])
            ones = sb.tile([1, 64], F32, name="ones")
            nc.vector.memset(ones[:], 1.0)
            sel = sb.tile([16, 4], F32, name="sel")
            nc.scalar.dma_start(sel[:], sel_in[:])

            # ---- fixed weight images (prefetchable, no deps) ------------
            awT = wfix.tile([128, 4096], BF16, name="awT")
            nc.sync.dma_start(awT[:], attn_wT_sh[:])
            encsb = wfix.tile([128, 4096], BF16, name="encsb")
            nc.sync.dma_start(encsb[:], enc_sh[:])
            cwT = wfix.tile([128, 3072], BF16, name="cwT")
            nc.sync.dma_start(cwT[:], comb_wT_sh[:])
            lw = wfix.tile([128, 16384], BF16, name="lw")
            nc.sync.dma_start(lw[:], lstm_w[:])

            # ---- phase A: embed gather, h0 columns, score partials ------
            xb = sb.tile([128, 1], I32, name="xb")
            nc.scalar.dma_start(xb[:], x128[:])
            idt = sb.tile([128, 2], I32, name="idt")
            nc.scalar.dma_start(idt[:], idxc[:])
            idx = sb.tile([128, 1], I32, name="idx")
            nc.vector.tensor_tensor(out=idx[:], in0=xb[:], in1=idt[:, 1:2], op=ALU.mult)
            nc.vector.tensor_tensor(out=idx[:], in0=idx[:], in1=idt[:, 0:1], op=ALU.add)
            embcol = sb.tile([128, 1], BF16, name="embcol")
            nc.gpsimd.indirect_dma_start(
                out=embcol[:], out_offset=None,
                in_=emb_sh[:],
                in_offset=bass.IndirectOffsetOnAxis(ap=idx[:, :1], axis=0),
            )

            h0sb = sb.tile([16, 128], F32, name="h0sb")
            nc.scalar.dma_start(h0sb[:], h0r[:])
            h0T_bf = sb.tile([128, 16], BF16, name="h0T_bf")
            h0fcol = sb.tile([128, 1], BF16, name="h0fcol")
            scores = sb.tile([1, L], F32, name="scores")
            with tc.tile_pool(name="psA", bufs=1, space="PSUM") as psA:
                h0T_ps = psA.tile([128, 16], F32, name="h0T_ps")
                nc.tensor.transpose(out=h0T_ps[:], in_=h0sb[:], identity=ident[:16, :16])
                nc.vector.tensor_copy(out=h0T_bf[:], in_=h0T_ps[:])
                h0f_ps = psA.tile([128, 1], F32, name="h0f_ps")
                nc.tensor.matmul(h0f_ps[:], lhsT=h0sb[:], rhs=sel[:, 0:1],
                                 start=True, stop=True)
                nc.vector.tensor_copy(out=h0fcol[:], in_=h0f_ps[:])

                for nt in range(4):
                    ps = psA.tile([1, 512], F32, name="ps_sc", tag="ps_sc", bufs=2)
                    o = nt * 512
                    nc.tensor.matmul(ps[:], lhsT=embcol[:], rhs=awT[:, o:o + 512],
                                     start=True, stop=False)
                    nc.tensor.matmul(ps[:], lhsT=h0fcol[:], rhs=awT[:, 2048 + o:2048 + o + 512],
                                     start=False, stop=True)
                    nc.scalar.copy(out=scores[:, o:o + 512], in_=ps[:])
            nc.scalar.dma_start(cc_sc_in[:], scores[:])
            nc.gpsimd.collective_compute(
                "AllReduce", ALU.add, ins=[cc_sc_in.opt()], outs=[cc_sc_out.opt()],
                replica_groups=[CORE_IDS])

            # ---- phase B: softmax + att_applied partial ------------------
            scf = sb.tile([16, 128], F32, name="scf")
            nc.scalar.dma_start(scf[:], cc_sc_out[:].rearrange("(r k) -> r k", r=16))
            ab2 = sb.tile([16, 128], F32, name="ab2")
            nc.scalar.dma_start(ab2[:], attn_b2[:])
            nc.vector.tensor_tensor(out=scf[:], in0=scf[:], in1=ab2[:], op=ALU.add)
            expsc = sb.tile([16, 128], F32, name="expsc")
            sums16 = sb.tile([16, 1], F32, name="sums16")
            nc.scalar.activation(out=expsc[:], in_=scf[:], func=AF.Exp, accum_out=sums16[:])
            sumrow = sb.tile([1, 16], F32, name="sumrow")
            denom = sb.tile([1, 1], F32, name="denom")
            recip = sb.tile([1, 1], F32, name="recip")
            rbc16 = sb.tile([16, 1], F32, name="rbc16")
            attw2d = sb.tile([16, 128], F32, name="attw2d")
            awcols_bf = sb.tile([128, 2], BF16, name="awcols_bf")
            aa_row = sb.tile([1, L], F32, name="aa_row")
            with tc.tile_pool(name="psB", bufs=1, space="PSUM") as psB:
                srp = psB.tile([1, 16], F32, name="srp")
                nc.tensor.transpose(out=srp[:], in_=sums16[:], identity=ident[:16, :16])
                nc.vector.tensor_copy(out=sumrow[:], in_=srp[:])
                nc.vector.reduce_sum(denom[:], sumrow[:], axis=AX.X)
                nc.vector.reciprocal(recip[:], denom[:])
                rrow = sb.tile([1, 16], F32, name="rrow")
                nc.vector.tensor_scalar_mul(rrow[:], ones[:, 0:16], recip[:])
                rbp = psB.tile([16, 1], F32, name="rbp")
                nc.tensor.transpose(out=rbp[:], in_=rrow[:], identity=ident[:1, :1])
                nc.vector.tensor_copy(out=rbc16[:], in_=rbp[:])
                nc.vector.tensor_scalar_mul(attw2d[:], expsc[:], rbc16[:])
                nc.scalar.dma_start(out_att[:].rearrange("(r k) -> r k", r=16), attw2d[:])
                # per-core att-weight columns via selector matmul
                awc_ps = psB.tile([128, 2], F32, name="awc_ps")
                nc.tensor.matmul(awc_ps[:], lhsT=attw2d[:], rhs=sel[:, 2:4],
                                 start=True, stop=True)
                nc.vector.tensor_copy(out=awcols_bf[:], in_=awc_ps[:])

                for nt in range(4):
                    ps = psB.tile([1, 512], F32, name="ps_aa", tag="ps_aa", bufs=2)
                    o = nt * 512
                    nc.tensor.matmul(ps[:], lhsT=awcols_bf[:, 0:1], rhs=encsb[:, o:o + 512],
                                     start=True, stop=False)
                    nc.tensor.matmul(ps[:], lhsT=awcols_bf[:, 1:2], rhs=encsb[:, 2048 + o:2048 + o + 512],
                                     start=False, stop=True)
                    nc.scalar.copy(out=aa_row[:, o:o + 512], in_=ps[:])
            nc.scalar.dma_start(cc_aa_in[:], aa_row[:])
            nc.gpsimd.collective_compute(
                "AllReduce", ALU.add, ins=[cc_aa_in.opt()], outs=[cc_aa_out.opt()],
                replica_groups=[CORE_IDS])

            # ---- phase C: combine matvec partial -------------------------
            aa2d = sb.tile([16, 128], F32, name="aa2d")
            nc.scalar.dma_start(aa2d[:], cc_aa_out[:].rearrange("(r k) -> r k", r=16))
            aacols_bf = sb.tile([128, 2], BF16, name="aacols_bf")
            li_row = sb.tile([1, H], F32, name="li_row")
            with tc.tile_pool(name="psC", bufs=1, space="PSUM") as psC:
                aac_ps = psC.tile([128, 2], F32, name="aac_ps")
                nc.tensor.matmul(aac_ps[:], lhsT=aa2d[:], rhs=sel[:, 2:4],
                                 start=True, stop=True)
                nc.vector.tensor_copy(out=aacols_bf[:], in_=aac_ps[:])
                for nt in range(2):
                    ps = psC.tile([1, 512], F32, name="ps_li", tag="ps_li", bufs=2)
                    o = nt * 512
                    nc.tensor.matmul(ps[:], lhsT=embcol[:], rhs=cwT[:, o:o + 512],
                                     start=True, stop=False)
                    nc.tensor.matmul(ps[:], lhsT=aacols_bf[:, 0:1], rhs=cwT[:, 1024 + o:1024 + o + 512],
                                     start=False, stop=False)
                    nc.tensor.matmul(ps[:], lhsT=aacols_bf[:, 1:2], rhs=cwT[:, 2048 + o:2048 + o + 512],
                                     start=False, stop=True)
                    nc.scalar.copy(out=li_row[:, o:o + 512], in_=ps[:])
            nc.scalar.dma_start(cc_li_in[:], li_row[:])
            nc.gpsimd.collective_compute(
                "AllReduce", ALU.add, ins=[cc_li_in.opt()], outs=[cc_li_out.opt()],
                replica_groups=[CORE_IDS])

            # ---- phase D: relu + LSTM cell (output-sharded) -------------
            li2d = sb.tile([8, 128], F32, name="li2d")
            nc.scalar.dma_start(li2d[:], cc_li_out[:].rearrange("(r k) -> r k", r=8))
            cb2 = sb.tile([8, 128], F32, name="cb2")
            nc.scalar.dma_start(cb2[:], comb_b2[:])
            nc.vector.tensor_tensor(out=li2d[:], in0=li2d[:], in1=cb2[:], op=ALU.add)
            nc.scalar.activation(out=li2d[:], in_=li2d[:], func=AF.Relu)
            licols_bf = sb.tile([128, 8], BF16, name="licols_bf")
            lb = sb.tile([1, 2048], F32, name="lb")
            nc.scalar.dma_start(lb[:], lstm_b[:])
            c0sb = sb.tile([1, 256], F32, name="c0sb")
            nc.scalar.dma_start(c0sb[:], c0_sh[:])
            gf = sb.tile([1, 512], F32, name="gf")
            gb = sb.tile([1, 512], F32, name="gb")
            hcrow = sb.tile([1, 512], F32, name="hcrow")
            with tc.tile_pool(name="psD", bufs=1, space="PSUM") as psD:
                li_ps = psD.tile([128, 8], F32, name="li_ps")
                nc.tensor.transpose(out=li_ps[:], in_=li2d[:], identity=ident[:8, :8])
                nc.vector.tensor_copy(out=licols_bf[:], in_=li_ps[:])

                # h0-dependent (early) matmuls first, lstm_in-dependent last
                ps_gf = psD.tile([1, 512], F32, name="ps_gf")
                ps_gb = psD.tile([1, 512], F32, name="ps_gb")
                for k in range(8):
                    nc.tensor.matmul(ps_gf[:], lhsT=h0T_bf[:, k:k + 1],
                                     rhs=lw[:, 4096 + k * 512:4096 + (k + 1) * 512],
                                     start=(k == 0), stop=False)
                for k in range(8):
                    nc.tensor.matmul(ps_gb[:], lhsT=h0T_bf[:, 8 + k:9 + k],
                                     rhs=lw[:, 12288 + k * 512:12288 + (k + 1) * 512],
                                     start=(k == 0), stop=False)
                for k in range(8):
                    nc.tensor.matmul(ps_gf[:], lhsT=licols_bf[:, k:k + 1],
                                     rhs=lw[:, k * 512:(k + 1) * 512],
                                     start=False, stop=(k == 7))
                for k in range(8):
                    nc.tensor.matmul(ps_gb[:], lhsT=licols_bf[:, k:k + 1],
                                     rhs=lw[:, 8192 + k * 512:8192 + (k + 1) * 512],
                                     start=False, stop=(k == 7))
                # gates = psum + b_ih + b_hh
                nc.vector.tensor_tensor(out=gf[:], in0=ps_gf[:], in1=lb[:, 0:512], op=ALU.add)
                nc.vector.tensor_tensor(out=gf[:], in0=gf[:], in1=lb[:, 512:1024], op=ALU.add)
                nc.vector.tensor_tensor(out=gb[:], in0=ps_gb[:], in1=lb[:, 1024:1536], op=ALU.add)
                nc.vector.tensor_tensor(out=gb[:], in0=gb[:], in1=lb[:, 1536:2048], op=ALU.add)

            for (g, coff, hoff) in ((gf, 0, 0), (gb, 128, 128)):
                sig_if = sb.tile([1, 256], F32, name=f"sig_if{hoff}")
                nc.scalar.activation(out=sig_if[:], in_=g[:, 0:256], func=AF.Sigmoid)
                sig_o = sb.tile([1, 128], F32, name=f"sig_o{hoff}")
                nc.scalar.activation(out=sig_o[:], in_=g[:, 384:512], func=AF.Sigmoid)
                tan_g = sb.tile([1, 128], F32, name=f"tan_g{hoff}")
                nc.scalar.activation(out=tan_g[:], in_=g[:, 256:384], func=AF.Tanh)
                c2 = sb.tile([1, 128], F32, name=f"c2_{hoff}")
                nc.vector.tensor_tensor(out=c2[:], in0=sig_if[:, 128:256],
                                        in1=c0sb[:, coff:coff + 128], op=ALU.mult)
                igg = sb.tile([1, 128], F32, name=f"igg{hoff}")
                nc.vector.tensor_tensor(out=igg[:], in0=sig_if[:, 0:128], in1=tan_g[:], op=ALU.mult)
                nc.vector.tensor_tensor(out=c2[:], in0=c2[:], in1=igg[:], op=ALU.add)
                tan_c2 = sb.tile([1, 128], F32, name=f"tan_c2{hoff}")
                nc.scalar.activation(out=tan_c2[:], in_=c2[:], func=AF.Tanh)
                h2 = sb.tile([1, 128], F32, name=f"h2_{hoff}")
                nc.vector.tensor_tensor(out=h2[:], in0=sig_o[:], in1=tan_c2[:], op=ALU.mult)
                nc.vector.tensor_copy(out=hcrow[:, hoff:hoff + 128], in_=h2[:])
                nc.vector.tensor_copy(out=hcrow[:, 256 + hoff:256 + hoff + 128], in_=c2[:])

            nc.scalar.dma_start(cc_hc_in[:], hcrow[:])
            nc.gpsimd.collective_compute(
                "AllGather", ALU.bypass, ins=[cc_hc_in.opt()], outs=[cc_hc_out.opt()],
                replica_groups=[CORE_IDS])

            # ---- phase E: output projection + distributed log-softmax ---
            nc.scalar.dma_start(out_hc[:], cc_hc_out[:])
            hc2d = sb.tile([8, 512], F32, name="hc2d")
            nc.scalar.dma_start(hc2d[:], cc_hc_out[:])
            locols_bf = sb.tile([128, 16], BF16, name="locols_bf")

            logits2d = sb.tile([52, 128], F32, name="logits2d")
            ob2 = sb.tile([52, 128], F32, name="ob2")
            nc.scalar.dma_start(ob2[:], outb_sh[:])
            with tc.tile_pool(name="psE", bufs=1, space="PSUM") as psE:
                lo_ps = psE.tile([128, 16], F32, name="lo_ps")
                nc.tensor.transpose(out=lo_ps[:, 0:8], in_=hc2d[:, 0:128],
                                    identity=ident[:8, :8])
                nc.tensor.transpose(out=lo_ps[:, 8:16], in_=hc2d[:, 128:256],
                                    identity=ident[:8, :8])
                nc.vector.tensor_copy(out=locols_bf[:], in_=lo_ps[:])
                for j in range(13):
                    wj = wbig.tile([128, 8192], BF16, name="wj", tag="wj", bufs=6)
                    nc.sync.dma_start(wj[:], wout[j])
                    ps = psE.tile([1, 512], F32, name="ps_lg", tag="ps_lg", bufs=2)
                    for k in range(16):
                        nc.tensor.matmul(ps[:], lhsT=locols_bf[:, k:k + 1],
                                         rhs=wj[:, k * 512:(k + 1) * 512],
                                         start=(k == 0), stop=(k == 15))
                    lrow = sb.tile([1, 512], F32, name="lrow", tag="lrow", bufs=2)
                    nc.scalar.copy(out=lrow[:], in_=ps[:])
                    nc.scalar.dma_start(logits2d[4 * j:4 * j + 4, :], lrow[:])

                nc.vector.tensor_tensor(out=logits2d[:], in0=logits2d[:], in1=ob2[:], op=ALU.add)
                exps = sb.tile([52, 128], F32, name="exps")
                sums52 = sb.tile([52, 1], F32, name="sums52")
                nc.scalar.activation(out=exps[:], in_=logits2d[:], func=AF.Exp,
                                     accum_out=sums52[:])
                s_row = sb.tile([1, 52], F32, name="s_row")
                srp2 = psE.tile([1, 52], F32, name="srp2")
                nc.tensor.transpose(out=srp2[:], in_=sums52[:], identity=ident[:52, :52])
                nc.vector.tensor_copy(out=s_row[:], in_=srp2[:])
                s_loc = sb.tile([1, 1], F32, name="s_loc")
                nc.vector.reduce_sum(s_loc[:], s_row[:], axis=AX.X)
                st_row = sb.tile([1, 8], F32, name="st_row")
                nc.vector.memset(st_row[:], 0.0)
                nc.vector.tensor_copy(out=st_row[:, 0:1], in_=s_loc[:])
                nc.scalar.dma_start(cc_st_in[:], st_row[:])
                nc.gpsimd.collective_compute(
                    "AllGather", ALU.bypass, ins=[cc_st_in.opt()], outs=[cc_st_out.opt()],
                    replica_groups=[CORE_IDS])

                stall = sb.tile([1, 64], F32, name="stall")
                nc.scalar.dma_start(stall[:], cc_st_out[:])
                s_glob = sb.tile([1, 1], F32, name="s_glob")
                nc.vector.reduce_sum(s_glob[:], stall[:, 0:64:8], axis=AX.X)
                nlz = sb.tile([1, 1], F32, name="nlz")
                nc.scalar.activation(out=nlz[:], in_=s_glob[:], func=AF.Ln)
                nc.scalar.mul(nlz[:], nlz[:], -1.0)
                nlzrow = sb.tile([1, 52], F32, name="nlzrow")
                nc.vector.tensor_scalar_mul(nlzrow[:], ones[:, 0:52], nlz[:])
                nlzc_ps = psE.tile([52, 1], F32, name="nlzc_ps")
                nc.tensor.transpose(out=nlzc_ps[:], in_=nlzrow[:], identity=ident[:1, :1])
                nlzcol = sb.tile([52, 1], F32, name="nlzcol")
                nc.vector.tensor_copy(out=nlzcol[:], in_=nlzc_ps[:])
                logp2d = sb.tile([52, 128], F32, name="logp2d")
                nc.scalar.activation(out=logp2d[:], in_=logits2d[:], func=AF.Identity,
                                     bias=nlzcol[:])
                nc.scalar.dma_start(out_logp[:].rearrange("(r k) -> r k", r=52), logp2d[:])

    if split:
        split_multiwait(nc, limit=1)
    return nc


def shard_inputs(x, h0, c0, encoder_outputs, emb, attn_w, attn_b, comb_w, comb_b,
                 w_ih_f, w_hh_f, b_ih_f, b_hh_f, w_ih_b, w_hh_b, b_ih_b, b_hh_b,
                 out_w, out_b):
    """Host-side sharding/packing. Returns list of per-core input dicts."""
    import ml_dtypes
    bf16 = ml_dtypes.bfloat16

    f32 = np.float32
    x = np.asarray(x).astype(np.int32).reshape(1)
    h0 = np.asarray(h0, f32)
    c0 = np.asarray(c0, f32)
    enc = np.asarray(encoder_outputs, f32)
    emb = np.asarray(emb, f32)
    attn_w = np.asarray(attn_w, f32)
    attn_b = np.asarray(attn_b, f32)
    comb_w = np.asarray(comb_w, f32)
    comb_b = np.asarray(comb_b, f32)
    out_w = np.asarray(out_w, f32)
    out_b = np.asarray(out_b, f32)

    x128 = np.full((128, 1), x[0], np.int32)
    idxc = np.stack([np.arange(128, dtype=np.int32),
                     np.full(128, 128, np.int32)], axis=1)  # [128,2]
    h0r = h0.reshape(2, 1024).reshape(16, 128)
    attn_b2 = attn_b.reshape(16, 128)
    comb_b2 = comb_b.reshape(8, 128)
    attn_wT = np.ascontiguousarray(attn_w.T)      # [2048, 2048] = [2H, L]
    comb_wT = np.ascontiguousarray(comb_w.T)      # [3072, 1024]
    w_ihT_f = w_ih_f.T                            # [1024, 4096]
    w_hhT_f = w_hh_f.T
    w_ihT_b = w_ih_b.T
    w_hhT_b = w_hh_b.T

    # padded vocab output projection
    wT_pad = np.zeros((2 * H, VP), f32)
    wT_pad[:, :V] = out_w.T
    outb_pad = np.full(VP, NEG, f32)
    outb_pad[:V] = out_b

    in_maps = []
    for c in range(NC):
        emb_c = np.ascontiguousarray(emb[:, c * 128:(c + 1) * 128]).astype(bf16)
        sel = np.zeros((16, 4), f32)
        sel[c, 0] = 1.0        # h0f slice selector
        sel[8 + c, 1] = 1.0    # h0b slice selector (unused)
        sel[2 * c, 2] = 1.0    # [c*256, c*256+128) row selector
        sel[2 * c + 1, 3] = 1.0

        # attention wT image: k0 = embed dims slice, k1 = h0f dims slice
        aw = np.empty((128, 4096), f32)
        aw[:, 0:2048] = attn_wT[c * 128:(c + 1) * 128, :]
        aw[:, 2048:4096] = attn_wT[H + c * 128:H + (c + 1) * 128, :]

        # encoder image: rows [c*256, c*256+256) as two k-chunks
        en = np.empty((128, 4096), f32)
        en[:, 0:2048] = enc[c * 256:c * 256 + 128, :]
        en[:, 2048:4096] = enc[c * 256 + 128:c * 256 + 256, :]

        # combine wT image: k0 = embed slice, k1/k2 = att slices
        cw = np.empty((128, 3072), f32)
        cw[:, 0:1024] = comb_wT[c * 128:(c + 1) * 128, :]
        cw[:, 1024:2048] = comb_wT[H + c * 256:H + c * 256 + 128, :]
        cw[:, 2048:3072] = comb_wT[H + c * 256 + 128:H + c * 256 + 256, :]

        # lstm weight image: per-core output columns (4 gate slices of 128)
        cols = np.concatenate([np.arange(g * H + c * 128, g * H + (c + 1) * 128)
                               for g in range(4)])
        lwimg = np.empty((128, 16384), f32)
        for mi, m in enumerate((w_ihT_f, w_hhT_f, w_ihT_b, w_hhT_b)):
            shard = m[:, cols]                    # [1024, 512]
            img = shard.reshape(8, 128, 512).transpose(1, 0, 2).reshape(128, 4096)
            lwimg[:, mi * 4096:(mi + 1) * 4096] = img
        lstm_b_row = np.concatenate([b_ih_f[cols], b_hh_f[cols],
                                     b_ih_b[cols], b_hh_b[cols]]).astype(f32)[None, :]

        # wout blocks: [13][128][16*512]; block j col (k*512+n) row p =
        # wT_pad[k*128+p, c*VPC + j*512 + n]
        wslab = wT_pad[:, c * VPC:(c + 1) * VPC]          # [2048, 6656]
        wj = wslab.reshape(16, 128, 13, 512).transpose(2, 1, 0, 3).reshape(13, 128, 8192)

        in_maps.append({
            "x128": x128,
            "idxc": idxc,
            "emb_sh": emb_c.reshape(V * 128, 1),
            "h0r": h0r,
            "c0_sh": np.concatenate([c0[0, 0, c * 128:(c + 1) * 128],
                                     c0[1, 0, c * 128:(c + 1) * 128]])[None, :],
            "sel_in": sel,
            "attn_wT_sh": aw.astype(bf16),
            "attn_b2": attn_b2,
            "enc_sh": en.astype(bf16),
            "comb_wT_sh": cw.astype(bf16),
            "comb_b2": comb_b2,
            "lstm_w": lwimg.astype(bf16),
            "lstm_b": lstm_b_row,
            "wout": np.ascontiguousarray(wj).astype(bf16),
            "outb_sh": outb_pad[c * VPC:(c + 1) * VPC].reshape(52, 128),
        })
    return in_maps


def assemble_outputs(results):
    logp = np.concatenate([np.asarray(results[c]["out_logp"]).reshape(-1)
                           for c in range(NC)])
    logp = logp[:V][None, :].astype(np.float32)
    att = np.asarray(results[0]["out_att"]).reshape(-1)[None, :].astype(np.float32)
    hc = np.asarray(results[0]["out_hc"]).reshape(8, 512)
    h_n = np.stack([hc[:, 0:128].reshape(1, H), hc[:, 128:256].reshape(1, H)])
    c_n = np.stack([hc[:, 256:384].reshape(1, H), hc[:, 384:512].reshape(1, H)])
    return logp, h_n.astype(np.float32), c_n.astype(np.float32), att


def kernel(**inputs):
    from concourse.bass_utils import run_bass_kernel_spmd
    nc = build_bass()
    in_maps = shard_inputs(**inputs)
    res = run_bass_kernel_spmd(nc, in_maps, CORE_IDS, trace=False)
    return assemble_outputs(res.results)
